# revision 27
# baseline (speedup 1.0000x reference)
"""Trainium2 Bass kernel for BertSelfAttention (B=1, S=4096, HID=768, 12 heads).

Sharding: 8 cores = 4 head-groups x 2 query-halves. Each core computes 3 heads
for 2048 query rows against all 4096 keys, fused (scores never hit HBM).

Host-side sharding prep packs each core's inputs in their on-chip layout
(bf16, transposed hidden states, chunk-major weights), so the device spends no
time on layout transforms.

Per-core dataflow (bf16 matmuls, fp32 PSUM accumulation):
  - score matmuls contract only HD=64 partitions and run PAIRWISE CONCURRENT
    on the PE via row tiling: heads 0/2 hold Q^T/K^T on partitions 0:64,
    head 1 (and a duplicate copy of head 2) on partitions 64:128. Each
    gg step issues tile_position (0,0) and (64,0) matmuls that execute
    simultaneously on complementary halves of the PE array.
  - paired projection matmuls produce two heads per instruction (head 0 cols
    0:64 + head 1 cols 64:128 of the stationary weights); head 2's unit
    carries [h2|h2] so both partition halves get a copy.
  - scores land transposed (S^T[k, q]) in PSUM; one ScalarE Exp per
    [128, 1024] tile writes bf16 P^T straight to SBUF (scale=1/8 folded in).
  - additive attention mask handled exactly by scaling V rows (and the
    appended ones-column) with exp(mask[k]) computed on device.
  - V is augmented with a ones column per head, so the context matmul
    accumulates both sum(p*v) and sum(p) (the softmax denominator) in one
    PSUM group.
  - ctx^T [65, 512] tiles are PE-transposed back to [q, d] layout, divided by
    the denominator on VectorE, and DMA'd out.
  - projection/V work is hand-interleaved into the attention sweep so the PE
    fills activation bubbles instead of serializing up front.
"""

import sys

sys.path.insert(0, "/opt/trn_rl_repo")

import ml_dtypes
import numpy as np

import concourse.bacc as bacc
import concourse.mybir as mybir
import concourse.tile as tile
from concourse import bass_utils

B, S, HID = 1, 4096, 768
NH, HD = 12, 64
N_CORES = 8
HG = 4  # head-groups (tensor parallel)
QS = 2  # query splits (data parallel on sequence)
HPC = NH // HG  # 3 heads per core
SQ = S // QS  # 2048 query rows per core
CC = HPC * HD  # 192 projection columns per core
WCC = 256  # weight cols per chunk in wqb/wkb: [h0|h1|h2|h2]
VC = HPC * (HD + 1)  # 195 augmented V columns (ones col per head)
NHC = HID // 128  # 6 contraction chunks
NT = S // 128  # 32 key tiles

f32 = mybir.dt.float32
bf16 = mybir.dt.bfloat16
bf16np = ml_dtypes.bfloat16

_CACHE = {}


def _build():
    EXP = mybir.ActivationFunctionType.Exp
    nc = bacc.Bacc("TRN2", target_bir_lowering=False)

    hsT_d = nc.dram_tensor("hsT", [HID, S], bf16, kind="ExternalInput")
    hsqT_d = nc.dram_tensor("hsqT", [HID, SQ], bf16, kind="ExternalInput")
    wqb_d = nc.dram_tensor("wqb", [128, NHC * WCC], bf16, kind="ExternalInput")
    wkb_d = nc.dram_tensor("wkb", [128, NHC * WCC], bf16, kind="ExternalInput")
    wvb_d = nc.dram_tensor("wvb", [128, NHC * VC], bf16, kind="ExternalInput")
    bqt_d = nc.dram_tensor("bqt", [128, HPC], f32, kind="ExternalInput")
    bkt_d = nc.dram_tensor("bkt", [128, HPC], f32, kind="ExternalInput")
    bvb_d = nc.dram_tensor("bvb", [128, VC], bf16, kind="ExternalInput")
    maskt_d = nc.dram_tensor("maskt", [128, NT], f32, kind="ExternalInput")
    ident_d = nc.dram_tensor("ident", [128, 128], f32, kind="ExternalInput")
    out_d = nc.dram_tensor("out", [SQ, CC], f32, kind="ExternalOutput")

    with tile.TileContext(nc) as tc:
        with (
            tc.tile_pool(name="persist", bufs=1) as P,
            tc.tile_pool(name="work", bufs=6) as WK,
            tc.tile_pool(name="outp", bufs=2) as OP,
            tc.tile_pool(name="ppsum", bufs=2, space="PSUM") as PP,
            tc.tile_pool(name="bpsum", bufs=2, space="PSUM") as BP,
            tc.tile_pool(name="cpsum", bufs=2, space="PSUM") as CP,
        ):
            # ---- persistent SBUF tensors ----
            # chunk-major transposed activations: chunk c at cols [c*S, (c+1)*S)
            hsT = P.tile([128, NHC * S], bf16, tag="hsT")
            hsTq = P.tile([128, NHC * SQ], bf16, tag="hsTq")
            wqb = P.tile([128, NHC * WCC], bf16, tag="wqb")
            wkb = P.tile([128, NHC * WCC], bf16, tag="wkb")
            wvb = P.tile([128, NHC * VC], bf16, tag="wvb")
            bvb = P.tile([128, VC], bf16, tag="bvb")
            bqt = P.tile([128, HPC], f32, tag="bqt")
            bkt = P.tile([128, HPC], f32, tag="bkt")
            maskt = P.tile([128, NT], f32, tag="maskt")
            wmask = P.tile([128, NT], f32, tag="wmask")
            identf = P.tile([128, 128], f32, tag="identf")
            # qt/kt partition halves: qt[0] lo=h0, qt[1] hi=h1, qt[2] both=h2
            qt = [
                P.tile([128, SQ], bf16, tag=f"qt{h}", name=f"qt{h}")
                for h in range(HPC)
            ]
            kt = [
                P.tile([128, S], bf16, tag=f"kt{h}", name=f"kt{h}")
                for h in range(HPC)
            ]
            vv = P.tile([128, NT * VC], bf16, tag="vv")

            # ---- emission helpers ----
            hsT_3d = hsT.rearrange("p (c s) -> p c s", s=S)
            hsT_d3 = hsT_d.rearrange("(c p) s -> p c s", p=128)
            hsTq_3d = hsTq.rearrange("p (c s) -> p c s", s=SQ)
            hsqT_d3 = hsqT_d.rearrange("(c p) s -> p c s", p=128)

            def load_hsT_cols(s0, s1):
                nc.sync.dma_start(hsT_3d[:, :, s0:s1], hsT_d3[:, :, s0:s1])

            def load_hsqT_cols(s0, s1):
                nc.sync.dma_start(hsTq_3d[:, :, s0:s1], hsqT_d3[:, :, s0:s1])

            # projection units: the h0/h1 pair shares one matmul chain
            # (stationary cols 0:128 of the chunk), h2 uses cols 128:256
            # ([h2|h2] duplicated, so both halves get a copy)
            def proj_writeback(kind, key, ps):
                dst = qt if kind == "qt" else kt
                bias = bqt if kind == "qt" else bkt
                j = key[1]
                if key[0] == 0:
                    nc.vector.tensor_scalar_add(
                        dst[0][0:64, j * 512 : (j + 1) * 512],
                        ps[0:64, :],
                        bias[0:64, 0:1],
                    )
                    nc.vector.tensor_scalar_add(
                        dst[1][64:128, j * 512 : (j + 1) * 512],
                        ps[64:128, :],
                        bias[64:128, 1:2],
                    )
                else:
                    nc.vector.tensor_scalar_add(
                        dst[2][:, j * 512 : (j + 1) * 512],
                        ps[:],
                        bias[:, 2:3],
                    )

            qt_done = set()

            def qt_unit(hkey, j):
                key = (hkey, j)
                if key in qt_done:
                    return
                qt_done.add(key)
                coff = 0 if hkey == 0 else 128
                pq = PP.tile([128, 512], f32, tag="proj", name="pq")
                for c in range(NHC):
                    nc.tensor.matmul(
                        pq[:],
                        wqb[:, c * WCC + coff : c * WCC + coff + 128],
                        hsTq[:, c * SQ + j * 512 : c * SQ + (j + 1) * 512],
                        start=(c == 0),
                        stop=(c == NHC - 1),
                    )
                proj_writeback("qt", key, pq)

            kt_done = set()

            def kt_unit(hkey, j):
                # produces key block [512j, 512(j+1)) for the h0/h1 pair or h2
                key = (hkey, j)
                if key in kt_done:
                    return
                kt_done.add(key)
                coff = 0 if hkey == 0 else 128
                pk = PP.tile([128, 512], f32, tag="proj", name="pk")
                for c in range(NHC):
                    nc.tensor.matmul(
                        pk[:],
                        wkb[:, c * WCC + coff : c * WCC + coff + 128],
                        hsT[:, c * S + j * 512 : c * S + (j + 1) * 512],
                        start=(c == 0),
                        stop=(c == NHC - 1),
                    )
                proj_writeback("kt", key, pk)

            def v_unit(t):
                # V projection; bias add + mask scale on the (idle) VectorE
                pv = PP.tile([128, VC], f32, tag="proj", name="pv")
                for c in range(NHC):
                    nc.tensor.matmul(
                        pv[:],
                        hsT[:, c * S + t * 128 : c * S + (t + 1) * 128],
                        wvb[:, c * VC : (c + 1) * VC],
                        start=(c == 0),
                        stop=(c == NHC - 1),
                    )
                vt = WK.tile([128, VC], bf16, tag="vtmp", name="vt", bufs=2)
                nc.vector.tensor_tensor(vt[:], pv[:], bvb[:], mybir.AluOpType.add)
                nc.vector.tensor_scalar_mul(
                    vv[:, t * VC : (t + 1) * VC], vt[:], wmask[:, t : t + 1]
                )

            # ---- ramp: pipelined input loads + first-needed projections ----
            # mask load + exp first: the ScalarE is in-order, so this tiny
            # ACTIVATE must clear the queue before the first score exp; its
            # DMA must not sit behind the big activation transfers
            nc.sync.dma_start(maskt[:], maskt_d[:])
            nc.scalar.activation(wmask[:], maskt[:], EXP)
            # HAM warmup: keep the PE busy through the input-DMA wait so the
            # clock gate is at 8/8 (2.4 GHz) when the first projections run;
            # without this the whole ramp executes at 1.2 GHz
            wsrc = WK.tile([128, 512], bf16, tag="wsrc", name="wsrc", bufs=1)
            nc.vector.memset(wsrc[:], 0.0)
            wps = PP.tile([128, 512], f32, tag="proj", name="wps")
            for _ in range(34):
                nc.tensor.matmul(wps[:], wsrc[:, 0:128], wsrc[:], start=True, stop=True)
            load_hsqT_cols(0, 512)  # enough for qt(*, 0)
            nc.sync.dma_start(wqb[:], wqb_d[:])
            nc.sync.dma_start(bqt[:], bqt_d[:])
            load_hsT_cols(0, 512)  # enough for kt(0, 0)
            nc.sync.dma_start(wkb[:], wkb_d[:])
            nc.sync.dma_start(bkt[:], bkt_d[:])
            qt_unit(0, 0)
            kt_unit(0, 0)
            load_hsT_cols(512, 2048)
            nc.sync.dma_start(wvb[:], wvb_d[:])
            nc.sync.dma_start(bvb[:], bvb_d[:])
            nc.sync.dma_start(identf[:], ident_d[:])
            load_hsT_cols(2048, 4096)
            load_hsqT_cols(512, SQ)

            # stepwise projection queues: one matmul per m-step so unit
            # bursts never overrun the per-step ScalarE slack
            qt_q = []
            kt_q = []

            def proj_step():
                q = qt_q if qt_q else kt_q
                if not q:
                    return
                st = q[0]
                c = st["step"]
                kind, key = st["kind"], st["key"]
                coff = 0 if key[0] == 0 else 128
                if c == 0:
                    st["ps"] = PP.tile([128, 512], f32, tag="proj", name="ps")
                ps = st["ps"]
                if kind == "qt":
                    nc.tensor.matmul(
                        ps[:],
                        wqb[:, c * WCC + coff : c * WCC + coff + 128],
                        hsTq[:, c * SQ + key[1] * 512 : c * SQ + (key[1] + 1) * 512],
                        start=(c == 0),
                        stop=(c == NHC - 1),
                    )
                else:
                    nc.tensor.matmul(
                        ps[:],
                        wkb[:, c * WCC + coff : c * WCC + coff + 128],
                        hsT[:, c * S + key[1] * 512 : c * S + (key[1] + 1) * 512],
                        start=(c == 0),
                        stop=(c == NHC - 1),
                    )
                if c == NHC - 1:
                    proj_writeback(kind, key, ps)
                    q.pop(0)
                    return
                st["step"] += 1

            def enqueue_qt(hkey, j):
                key = (hkey, j)
                if key in qt_done:
                    return
                qt_done.add(key)
                qt_q.append({"kind": "qt", "key": key, "step": 0})

            def enqueue_kt(hkey, j):
                key = (hkey, j)
                if key in kt_done:
                    return
                kt_done.add(key)
                kt_q.append({"kind": "kt", "key": key, "step": 0})

            # deferred out-stage, pipelined into the next block's m-loop
            out_stage_q = []

            def out_stage_copies():
                # front-load the PSUM-freeing cs copies for every queued
                # entry so the next block's ctx chains (which reuse the CP
                # banks) depend on already-emitted DVE work
                for ent in out_stage_q:
                    st = ent[3]
                    if st["step"] == 0:
                        cs = OP.tile([65, 512], f32, tag="cs", name="cs")
                        nc.vector.tensor_copy(cs[:], ent[2][:])
                        st["cs"] = cs
                        st["ot"] = OP.tile([128, 4 * 64], f32, tag="ot", name="ot")
                        st["step"] = 1

            def emit_out_stage():
                if not out_stage_q:
                    return
                jq, h, cx, st = out_stage_q[0]
                if st["step"] == 0:
                    cs = OP.tile([65, 512], f32, tag="cs", name="cs")
                    nc.vector.tensor_copy(cs[:], cx[:])
                    st["cs"] = cs
                    st["ot"] = OP.tile([128, 4 * 64], f32, tag="ot", name="ot")
                elif st["step"] == 1:
                    # all four transposes back-to-back
                    cs = st["cs"]
                    tp2 = PP.tile([128, 4 * 65], f32, tag="proj", name="tp2")
                    st["tp2"] = tp2
                    for t4 in range(4):
                        nc.tensor.transpose(
                            tp2[:, t4 * 65 : (t4 + 1) * 65],
                            cs[:, t4 * 128 : (t4 + 1) * 128],
                            identf[0:65, 0:65],
                        )
                elif st["step"] <= 5:
                    t4 = st["step"] - 2
                    tp2, ot = st["tp2"], st["ot"]
                    if t4 == 0:
                        # one batched reciprocal over the four denominator
                        # columns (strided view) instead of four tiny ones
                        rc = OP.tile([128, 4], f32, tag="rc", name="rc")
                        st["rc"] = rc
                        nc.vector.reciprocal(
                            rc[:],
                            tp2.rearrange("p (t c) -> p t c", c=65)[:, :, 64],
                        )
                    rc = st["rc"]
                    nc.vector.tensor_scalar_mul(
                        ot[:, t4 * 64 : (t4 + 1) * 64],
                        tp2[:, t4 * 65 : t4 * 65 + 64],
                        rc[:, t4 : t4 + 1],
                    )
                    # per-chunk DMA: spreads the writeback across queues and
                    # shrinks the post-compute tail to one 32KB transfer
                    nc.sync.dma_start(
                        out_d[
                            jq * 512 + t4 * 128 : jq * 512 + (t4 + 1) * 128,
                            h * 64 : (h + 1) * 64,
                        ],
                        ot[:, t4 * 64 : (t4 + 1) * 64],
                    )
                    if t4 == 3:
                        out_stage_q.pop(0)
                        return
                st["step"] += 1

            def flush_out_stages():
                while out_stage_q:
                    emit_out_stage()

            # ---- attention sweep over head-pair blocks ----
            # pr=0: heads (0,1) paired across partition halves, 32 m-steps
            #       (one key tile per head per step)
            # pr=2: head 2 paired with its own duplicate, 16 m-steps
            #       (key tiles 2m / 2m+1)
            blocks = [(jq, pr) for pr in (0, 2) for jq in range(SQ // 512)]
            pending_final = None

            for bi, (jq, pr) in enumerate(blocks):
                qt_unit(pr, jq)
                nm = 32 if pr == 0 else 16
                if pr == 0:
                    cxs = [
                        CP.tile([65, 512], f32, tag="ctx", name=f"cx{bi}_0"),
                        CP.tile([65, 512], f32, tag="ctx", name=f"cx{bi}_1"),
                    ]
                    heads = (0, 1)
                else:
                    cxs = [CP.tile([65, 512], f32, tag="ctx", name=f"cx{bi}_2")]
                    heads = (2,)
                pts = []

                def emit_ctx(pm, cxs=cxs, pts=pts, pr=pr, nm=nm):
                    pt = pts[pm]
                    if pr == 0:
                        for hi, h in enumerate((0, 1)):
                            nc.tensor.matmul(
                                cxs[hi][:],
                                vv[:, pm * VC + h * 65 : pm * VC + h * 65 + 65],
                                pt[:, hi * 512 : (hi + 1) * 512],
                                start=(pm == 0),
                                stop=(pm == nm - 1),
                            )
                    else:
                        for gi, g in enumerate((2 * pm, 2 * pm + 1)):
                            nc.tensor.matmul(
                                cxs[0][:],
                                vv[:, g * VC + 2 * 65 : g * VC + 2 * 65 + 65],
                                pt[:, gi * 512 : (gi + 1) * 512],
                                start=(g == 0),
                                stop=(g == NT - 1),
                            )

                for m in range(nm):
                    # paired score matmuls: tile (0,0) on partitions 0:64 and
                    # tile (64,0) on partitions 64:128 run concurrently
                    sc = BP.tile([128, 1024], f32, tag="big", name="sc")
                    if pr == 0:
                        ga, gb = m, m
                        lo_t, hi_t = kt[0], kt[1]
                        lo_q, hi_q = qt[0], qt[1]
                    else:
                        ga, gb = 2 * m, 2 * m + 1
                        lo_t = hi_t = kt[2]
                        lo_q = hi_q = qt[2]
                    nc.tensor.matmul(
                        sc[:, 0:512],
                        lo_t[0:64, ga * 128 : (ga + 1) * 128],
                        lo_q[0:64, jq * 512 : (jq + 1) * 512],
                        start=True,
                        stop=True,
                    )
                    nc.tensor.matmul(
                        sc[:, 512:1024],
                        hi_t[64:128, gb * 128 : (gb + 1) * 128],
                        hi_q[64:128, jq * 512 : (jq + 1) * 512],
                        start=True,
                        stop=True,
                    )
                    pt = WK.tile([128, 1024], bf16, tag="pts", name="pt")
                    nc.scalar.activation(pt[:], sc[:], EXP, scale=0.125)
                    pts.append(pt)
                    if m == 0:
                        if pending_final is not None:
                            pending_final()
                            pending_final = None
                        out_stage_copies()
                    emit_out_stage()
                    # interleave remaining projection work into the
                    # activation-bound steady state (after the exp emission so
                    # scores are never delayed behind projection work)
                    if bi == 0:
                        v_unit(m)
                        if m % 4 == 0 and m // 4 + 1 <= 7:
                            kt_unit(0, m // 4 + 1)
                        # pre-stage the next blocks' q projections so block
                        # transitions never burst 6 matmuls before scores
                        if m == 24:
                            enqueue_qt(*reversed(blocks[1]))
                        if m >= 24:
                            proj_step()
                    else:
                        if m == 0 and bi == 1:
                            for j2 in range(8):
                                enqueue_kt(2, j2)
                        if m == (12 if nm == 32 else 8) and bi + 1 < len(blocks):
                            njq, npr = blocks[bi + 1]
                            enqueue_qt(npr, njq)
                        if m == (20 if nm == 32 else 10) and bi + 2 < len(blocks):
                            njq, npr = blocks[bi + 2]
                            enqueue_qt(npr, njq)
                        proj_step()

                    # ctx runs one step behind exp so the PE overlaps the
                    # activation latency with the previous step's ctx
                    if m > 0:
                        emit_ctx(m - 1)
                # final step's ctx is deferred into the next block so
                # the transition never stalls on the last exp
                pending_final = (lambda f=emit_ctx, n=nm: f(n - 1))
                for hi, h in enumerate(heads):
                    out_stage_q.append((jq, h, cxs[hi], {"step": 0}))
            if pending_final is not None:
                pending_final()
                pending_final = None
            flush_out_stages()

    nc.compile()
    return nc


def _get_nc():
    if "nc" not in _CACHE:
        _CACHE["nc"] = _build()
    return _CACHE["nc"]


def _in_maps(hs, mask, Wq, bq, Wk, bk, Wv, bv):
    ident = np.eye(128, dtype=np.float32)
    maskt = np.ascontiguousarray(mask.reshape(NT, 128).T)  # [128, 32]
    hsT = np.ascontiguousarray(hs.astype(bf16np).T)  # [768, 4096] bf16
    hsqT = [
        np.ascontiguousarray(hs[sh * SQ : (sh + 1) * SQ, :].astype(bf16np).T)
        for sh in range(QS)
    ]

    def qk_chunks(W, hg):  # [768, :] f32 -> [128, 6*256] bf16: [h0|h1|h2|h2]
        out = np.zeros((128, NHC * WCC), bf16np)
        for c in range(NHC):
            blk = W[c * 128 : (c + 1) * 128, hg * CC : (hg + 1) * CC].astype(
                bf16np
            )
            out[:, c * WCC : c * WCC + CC] = blk
            out[:, c * WCC + CC : c * WCC + 256] = blk[:, 128:192]
        return out

    def v_chunks(W):  # augmented V weights -> [128, 6*195] bf16
        out = np.empty((128, NHC * VC), bf16np)
        for c in range(NHC):
            out[:, c * VC : (c + 1) * VC] = W[c * 128 : (c + 1) * 128, :].astype(
                bf16np
            )
        return out

    maps = []
    for core in range(N_CORES):
        hg, sh = core // QS, core % QS
        wv_aug = np.zeros((HID, VC), np.float32)
        bv_aug = np.zeros((1, VC), np.float32)
        for h in range(HPC):
            wv_aug[:, h * 65 : h * 65 + 64] = Wv[
                :, hg * CC + h * 64 : hg * CC + (h + 1) * 64
            ]
            bv_aug[0, h * 65 : h * 65 + 64] = bv[
                hg * CC + h * 64 : hg * CC + (h + 1) * 64
            ]
            bv_aug[0, h * 65 + 64] = 1.0
        # per-head bias columns on each head's partition half (h2 on both)
        bqt = np.zeros((128, HPC), np.float32)
        bkt = np.zeros((128, HPC), np.float32)
        for h, lo in ((0, 0), (1, 64)):
            bqt[lo : lo + 64, h] = bq[hg * CC + h * 64 : hg * CC + (h + 1) * 64]
            bkt[lo : lo + 64, h] = bk[hg * CC + h * 64 : hg * CC + (h + 1) * 64]
        for lo in (0, 64):
            bqt[lo : lo + 64, 2] = bq[hg * CC + 128 : hg * CC + 192]
            bkt[lo : lo + 64, 2] = bk[hg * CC + 128 : hg * CC + 192]
        maps.append(
            {
                "hsT": hsT,
                "hsqT": hsqT[sh],
                "wqb": qk_chunks(Wq, hg),
                "wkb": qk_chunks(Wk, hg),
                "wvb": v_chunks(wv_aug),
                "bqt": bqt,
                "bkt": bkt,
                "bvb": np.ascontiguousarray(
                    np.broadcast_to(bv_aug.astype(bf16np), (128, VC))
                ),
                "maskt": maskt,
                "ident": ident,
            }
        )
    return maps


def kernel(hidden_states, attention_mask, Wq, bq, Wk, bk, Wv, bv, **run_kwargs):
    hs = np.ascontiguousarray(np.asarray(hidden_states, np.float32).reshape(S, HID))
    mask = np.ascontiguousarray(np.asarray(attention_mask, np.float32).reshape(S))
    Wq = np.asarray(Wq, np.float32)
    Wk = np.asarray(Wk, np.float32)
    Wv = np.asarray(Wv, np.float32)
    bq = np.asarray(bq, np.float32)
    bk = np.asarray(bk, np.float32)
    bv = np.asarray(bv, np.float32)

    nc = _get_nc()
    maps = _in_maps(hs, mask, Wq, bq, Wk, bk, Wv, bv)
    res = bass_utils.run_bass_kernel_spmd(
        nc, maps, core_ids=list(range(N_CORES)), **run_kwargs
    )
    out = np.zeros((S, NH * HD), np.float32)
    for core in range(N_CORES):
        hg, sh = core // QS, core % QS
        out[sh * SQ : (sh + 1) * SQ, hg * CC : (hg + 1) * CC] = res.results[core][
            "out"
        ]
    if "trace" in run_kwargs:
        _CACHE["last_result"] = res
    return out.reshape(B, S, NH * HD)


# revision 28
# speedup vs baseline: 1.0067x; 1.0067x over previous
"""Trainium2 Bass kernel for BertSelfAttention (B=1, S=4096, HID=768, 12 heads).

Sharding: 8 cores = 4 head-groups x 2 query-halves. Each core computes 3 heads
for 2048 query rows against all 4096 keys, fused (scores never hit HBM).

Host-side sharding prep packs each core's inputs in their on-chip layout
(bf16, transposed hidden states, chunk-major weights), so the device spends no
time on layout transforms.

Per-core dataflow (bf16 matmuls, fp32 PSUM accumulation):
  - score matmuls contract only HD=64 partitions and run PAIRWISE CONCURRENT
    on the PE via row tiling: heads 0/2 hold Q^T/K^T on partitions 0:64,
    head 1 (and a duplicate copy of head 2) on partitions 64:128. Each
    gg step issues tile_position (0,0) and (64,0) matmuls that execute
    simultaneously on complementary halves of the PE array.
  - paired projection matmuls produce two heads per instruction (head 0 cols
    0:64 + head 1 cols 64:128 of the stationary weights); head 2's unit
    carries [h2|h2] so both partition halves get a copy.
  - scores land transposed (S^T[k, q]) in PSUM; one ScalarE Exp per
    [128, 1024] tile writes bf16 P^T straight to SBUF (scale=1/8 folded in).
  - additive attention mask handled exactly by scaling V rows (and the
    appended ones-column) with exp(mask[k]) computed on device.
  - V is augmented with a ones column per head, so the context matmul
    accumulates both sum(p*v) and sum(p) (the softmax denominator) in one
    PSUM group.
  - ctx^T [65, 512] tiles are PE-transposed back to [q, d] layout, divided by
    the denominator on VectorE, and DMA'd out.
  - projection/V work is hand-interleaved into the attention sweep so the PE
    fills activation bubbles instead of serializing up front.
"""

import sys

sys.path.insert(0, "/opt/trn_rl_repo")

import ml_dtypes
import numpy as np

import concourse.bacc as bacc
import concourse.mybir as mybir
import concourse.tile as tile
from concourse import bass_utils

B, S, HID = 1, 4096, 768
NH, HD = 12, 64
N_CORES = 8
HG = 4  # head-groups (tensor parallel)
QS = 2  # query splits (data parallel on sequence)
HPC = NH // HG  # 3 heads per core
SQ = S // QS  # 2048 query rows per core
CC = HPC * HD  # 192 projection columns per core
WCC = 256  # weight cols per chunk in wqb/wkb: [h0|h1|h2|h2]
VC = HPC * (HD + 1)  # 195 augmented V columns (ones col per head)
NHC = HID // 128  # 6 contraction chunks
NT = S // 128  # 32 key tiles

f32 = mybir.dt.float32
bf16 = mybir.dt.bfloat16
bf16np = ml_dtypes.bfloat16

_CACHE = {}


def _build():
    EXP = mybir.ActivationFunctionType.Exp
    nc = bacc.Bacc("TRN2", target_bir_lowering=False)

    hsT_d = nc.dram_tensor("hsT", [HID, S], bf16, kind="ExternalInput")
    hsqT_d = nc.dram_tensor("hsqT", [HID, SQ], bf16, kind="ExternalInput")
    wqb_d = nc.dram_tensor("wqb", [128, NHC * WCC], bf16, kind="ExternalInput")
    wkb_d = nc.dram_tensor("wkb", [128, NHC * WCC], bf16, kind="ExternalInput")
    wvb_d = nc.dram_tensor("wvb", [128, NHC * VC], bf16, kind="ExternalInput")
    bqt_d = nc.dram_tensor("bqt", [128, HPC], f32, kind="ExternalInput")
    bkt_d = nc.dram_tensor("bkt", [128, HPC], f32, kind="ExternalInput")
    bvb_d = nc.dram_tensor("bvb", [128, VC], bf16, kind="ExternalInput")
    maskt_d = nc.dram_tensor("maskt", [128, NT], f32, kind="ExternalInput")
    ident_d = nc.dram_tensor("ident", [128, 128], f32, kind="ExternalInput")
    out_d = nc.dram_tensor("out", [SQ, CC], f32, kind="ExternalOutput")

    with tile.TileContext(nc) as tc:
        with (
            tc.tile_pool(name="persist", bufs=1) as P,
            tc.tile_pool(name="work", bufs=6) as WK,
            tc.tile_pool(name="outp", bufs=2) as OP,
            tc.tile_pool(name="ppsum", bufs=2, space="PSUM") as PP,
            tc.tile_pool(name="bpsum", bufs=2, space="PSUM") as BP,
            tc.tile_pool(name="cpsum", bufs=2, space="PSUM") as CP,
        ):
            # ---- persistent SBUF tensors ----
            # chunk-major transposed activations: chunk c at cols [c*S, (c+1)*S)
            hsT = P.tile([128, NHC * S], bf16, tag="hsT")
            hsTq = P.tile([128, NHC * SQ], bf16, tag="hsTq")
            wqb = P.tile([128, NHC * WCC], bf16, tag="wqb")
            wkb = P.tile([128, NHC * WCC], bf16, tag="wkb")
            wvb = P.tile([128, NHC * VC], bf16, tag="wvb")
            bvb = P.tile([128, VC], bf16, tag="bvb")
            bqt = P.tile([128, HPC], f32, tag="bqt")
            bkt = P.tile([128, HPC], f32, tag="bkt")
            maskt = P.tile([128, NT], f32, tag="maskt")
            wmask = P.tile([128, NT], f32, tag="wmask")
            identf = P.tile([128, 128], f32, tag="identf")
            # qt/kt partition halves: qt[0] lo=h0, qt[1] hi=h1, qt[2] both=h2
            qt = [
                P.tile([128, SQ], bf16, tag=f"qt{h}", name=f"qt{h}")
                for h in range(HPC)
            ]
            kt = [
                P.tile([128, S], bf16, tag=f"kt{h}", name=f"kt{h}")
                for h in range(HPC)
            ]
            vv = P.tile([128, NT * VC], bf16, tag="vv")

            # ---- emission helpers ----
            hsT_3d = hsT.rearrange("p (c s) -> p c s", s=S)
            hsT_d3 = hsT_d.rearrange("(c p) s -> p c s", p=128)
            hsTq_3d = hsTq.rearrange("p (c s) -> p c s", s=SQ)
            hsqT_d3 = hsqT_d.rearrange("(c p) s -> p c s", p=128)

            def load_hsT_cols(s0, s1):
                nc.sync.dma_start(hsT_3d[:, :, s0:s1], hsT_d3[:, :, s0:s1])

            def load_hsqT_cols(s0, s1):
                nc.sync.dma_start(hsTq_3d[:, :, s0:s1], hsqT_d3[:, :, s0:s1])

            # projection units: the h0/h1 pair shares one matmul chain
            # (stationary cols 0:128 of the chunk), h2 uses cols 128:256
            # ([h2|h2] duplicated, so both halves get a copy)
            def proj_writeback(kind, key, ps):
                dst = qt if kind == "qt" else kt
                bias = bqt if kind == "qt" else bkt
                j = key[1]
                if key[0] == 0:
                    nc.vector.tensor_scalar_add(
                        dst[0][0:64, j * 512 : (j + 1) * 512],
                        ps[0:64, :],
                        bias[0:64, 0:1],
                    )
                    nc.vector.tensor_scalar_add(
                        dst[1][64:128, j * 512 : (j + 1) * 512],
                        ps[64:128, :],
                        bias[64:128, 1:2],
                    )
                else:
                    nc.vector.tensor_scalar_add(
                        dst[2][:, j * 512 : (j + 1) * 512],
                        ps[:],
                        bias[:, 2:3],
                    )

            qt_done = set()

            def qt_unit(hkey, j):
                key = (hkey, j)
                if key in qt_done:
                    return
                qt_done.add(key)
                coff = 0 if hkey == 0 else 128
                pq = PP.tile([128, 512], f32, tag="proj", name="pq")
                for c in range(NHC):
                    nc.tensor.matmul(
                        pq[:],
                        wqb[:, c * WCC + coff : c * WCC + coff + 128],
                        hsTq[:, c * SQ + j * 512 : c * SQ + (j + 1) * 512],
                        start=(c == 0),
                        stop=(c == NHC - 1),
                    )
                proj_writeback("qt", key, pq)

            kt_done = set()

            def kt_unit(hkey, j):
                # produces key block [512j, 512(j+1)) for the h0/h1 pair or h2
                key = (hkey, j)
                if key in kt_done:
                    return
                kt_done.add(key)
                coff = 0 if hkey == 0 else 128
                pk = PP.tile([128, 512], f32, tag="proj", name="pk")
                for c in range(NHC):
                    nc.tensor.matmul(
                        pk[:],
                        wkb[:, c * WCC + coff : c * WCC + coff + 128],
                        hsT[:, c * S + j * 512 : c * S + (j + 1) * 512],
                        start=(c == 0),
                        stop=(c == NHC - 1),
                    )
                proj_writeback("kt", key, pk)

            def v_unit(t):
                # V projection; bias add + mask scale on the (idle) VectorE
                pv = PP.tile([128, VC], f32, tag="proj", name="pv")
                for c in range(NHC):
                    nc.tensor.matmul(
                        pv[:],
                        hsT[:, c * S + t * 128 : c * S + (t + 1) * 128],
                        wvb[:, c * VC : (c + 1) * VC],
                        start=(c == 0),
                        stop=(c == NHC - 1),
                    )
                vt = WK.tile([128, VC], bf16, tag="vtmp", name="vt", bufs=2)
                nc.vector.tensor_tensor(vt[:], pv[:], bvb[:], mybir.AluOpType.add)
                nc.vector.tensor_scalar_mul(
                    vv[:, t * VC : (t + 1) * VC], vt[:], wmask[:, t : t + 1]
                )

            # ---- ramp: pipelined input loads + first-needed projections ----
            # mask load + exp first: the ScalarE is in-order, so this tiny
            # ACTIVATE must clear the queue before the first score exp; its
            # DMA must not sit behind the big activation transfers
            nc.sync.dma_start(maskt[:], maskt_d[:])
            nc.scalar.activation(wmask[:], maskt[:], EXP)
            load_hsqT_cols(0, 512)  # enough for qt(*, 0)
            nc.sync.dma_start(wqb[:], wqb_d[:])
            nc.sync.dma_start(bqt[:], bqt_d[:])
            load_hsT_cols(0, 512)  # enough for kt(0, 0)
            nc.sync.dma_start(wkb[:], wkb_d[:])
            nc.sync.dma_start(bkt[:], bkt_d[:])
            qt_unit(0, 0)
            kt_unit(0, 0)
            load_hsT_cols(512, 2048)
            nc.sync.dma_start(wvb[:], wvb_d[:])
            nc.sync.dma_start(bvb[:], bvb_d[:])
            nc.sync.dma_start(identf[:], ident_d[:])
            load_hsT_cols(2048, 4096)
            load_hsqT_cols(512, SQ)

            # stepwise projection queues: one matmul per m-step so unit
            # bursts never overrun the per-step ScalarE slack
            qt_q = []
            kt_q = []

            def proj_step():
                q = qt_q if qt_q else kt_q
                if not q:
                    return
                st = q[0]
                c = st["step"]
                kind, key = st["kind"], st["key"]
                coff = 0 if key[0] == 0 else 128
                if c == 0:
                    st["ps"] = PP.tile([128, 512], f32, tag="proj", name="ps")
                ps = st["ps"]
                if kind == "qt":
                    nc.tensor.matmul(
                        ps[:],
                        wqb[:, c * WCC + coff : c * WCC + coff + 128],
                        hsTq[:, c * SQ + key[1] * 512 : c * SQ + (key[1] + 1) * 512],
                        start=(c == 0),
                        stop=(c == NHC - 1),
                    )
                else:
                    nc.tensor.matmul(
                        ps[:],
                        wkb[:, c * WCC + coff : c * WCC + coff + 128],
                        hsT[:, c * S + key[1] * 512 : c * S + (key[1] + 1) * 512],
                        start=(c == 0),
                        stop=(c == NHC - 1),
                    )
                if c == NHC - 1:
                    proj_writeback(kind, key, ps)
                    q.pop(0)
                    return
                st["step"] += 1

            def enqueue_qt(hkey, j):
                key = (hkey, j)
                if key in qt_done:
                    return
                qt_done.add(key)
                qt_q.append({"kind": "qt", "key": key, "step": 0})

            def enqueue_kt(hkey, j):
                key = (hkey, j)
                if key in kt_done:
                    return
                kt_done.add(key)
                kt_q.append({"kind": "kt", "key": key, "step": 0})

            # deferred out-stage, pipelined into the next block's m-loop
            out_stage_q = []

            def out_stage_copies():
                # front-load the PSUM-freeing cs copies for every queued
                # entry so the next block's ctx chains (which reuse the CP
                # banks) depend on already-emitted DVE work
                for ent in out_stage_q:
                    st = ent[3]
                    if st["step"] == 0:
                        cs = OP.tile([65, 512], f32, tag="cs", name="cs")
                        nc.vector.tensor_copy(cs[:], ent[2][:])
                        st["cs"] = cs
                        st["ot"] = OP.tile([128, 4 * 64], f32, tag="ot", name="ot")
                        st["step"] = 1

            def emit_out_stage():
                if not out_stage_q:
                    return
                jq, h, cx, st = out_stage_q[0]
                if st["step"] == 0:
                    cs = OP.tile([65, 512], f32, tag="cs", name="cs")
                    nc.vector.tensor_copy(cs[:], cx[:])
                    st["cs"] = cs
                    st["ot"] = OP.tile([128, 4 * 64], f32, tag="ot", name="ot")
                elif st["step"] == 1:
                    # all four transposes back-to-back
                    cs = st["cs"]
                    tp2 = PP.tile([128, 4 * 65], f32, tag="proj", name="tp2")
                    st["tp2"] = tp2
                    for t4 in range(4):
                        nc.tensor.transpose(
                            tp2[:, t4 * 65 : (t4 + 1) * 65],
                            cs[:, t4 * 128 : (t4 + 1) * 128],
                            identf[0:65, 0:65],
                        )
                elif st["step"] <= 5:
                    t4 = st["step"] - 2
                    tp2, ot = st["tp2"], st["ot"]
                    if t4 == 0:
                        # one batched reciprocal over the four denominator
                        # columns (strided view) instead of four tiny ones
                        rc = OP.tile([128, 4], f32, tag="rc", name="rc")
                        st["rc"] = rc
                        nc.vector.reciprocal(
                            rc[:],
                            tp2.rearrange("p (t c) -> p t c", c=65)[:, :, 64],
                        )
                    rc = st["rc"]
                    nc.vector.tensor_scalar_mul(
                        ot[:, t4 * 64 : (t4 + 1) * 64],
                        tp2[:, t4 * 65 : t4 * 65 + 64],
                        rc[:, t4 : t4 + 1],
                    )
                    # per-chunk DMA: spreads the writeback across queues and
                    # shrinks the post-compute tail to one 32KB transfer
                    nc.sync.dma_start(
                        out_d[
                            jq * 512 + t4 * 128 : jq * 512 + (t4 + 1) * 128,
                            h * 64 : (h + 1) * 64,
                        ],
                        ot[:, t4 * 64 : (t4 + 1) * 64],
                    )
                    if t4 == 3:
                        out_stage_q.pop(0)
                        return
                st["step"] += 1

            def flush_out_stages():
                while out_stage_q:
                    emit_out_stage()

            # ---- attention sweep over head-pair blocks ----
            # pr=0: heads (0,1) paired across partition halves, 32 m-steps
            #       (one key tile per head per step)
            # pr=2: head 2 paired with its own duplicate, 16 m-steps
            #       (key tiles 2m / 2m+1)
            blocks = [(jq, pr) for pr in (0, 2) for jq in range(SQ // 512)]
            pending_final = None

            for bi, (jq, pr) in enumerate(blocks):
                qt_unit(pr, jq)
                nm = 32 if pr == 0 else 16
                if pr == 0:
                    cxs = [
                        CP.tile([65, 512], f32, tag="ctx", name=f"cx{bi}_0"),
                        CP.tile([65, 512], f32, tag="ctx", name=f"cx{bi}_1"),
                    ]
                    heads = (0, 1)
                else:
                    cxs = [CP.tile([65, 512], f32, tag="ctx", name=f"cx{bi}_2")]
                    heads = (2,)
                pts = []

                def emit_ctx(pm, cxs=cxs, pts=pts, pr=pr, nm=nm):
                    pt = pts[pm]
                    if pr == 0:
                        for hi, h in enumerate((0, 1)):
                            nc.tensor.matmul(
                                cxs[hi][:],
                                vv[:, pm * VC + h * 65 : pm * VC + h * 65 + 65],
                                pt[:, hi * 512 : (hi + 1) * 512],
                                start=(pm == 0),
                                stop=(pm == nm - 1),
                            )
                    else:
                        for gi, g in enumerate((2 * pm, 2 * pm + 1)):
                            nc.tensor.matmul(
                                cxs[0][:],
                                vv[:, g * VC + 2 * 65 : g * VC + 2 * 65 + 65],
                                pt[:, gi * 512 : (gi + 1) * 512],
                                start=(g == 0),
                                stop=(g == NT - 1),
                            )

                for m in range(nm):
                    # paired score matmuls: tile (0,0) on partitions 0:64 and
                    # tile (64,0) on partitions 64:128 run concurrently
                    sc = BP.tile([128, 1024], f32, tag="big", name="sc")
                    if pr == 0:
                        ga, gb = m, m
                        lo_t, hi_t = kt[0], kt[1]
                        lo_q, hi_q = qt[0], qt[1]
                    else:
                        ga, gb = 2 * m, 2 * m + 1
                        lo_t = hi_t = kt[2]
                        lo_q = hi_q = qt[2]
                    nc.tensor.matmul(
                        sc[:, 0:512],
                        lo_t[0:64, ga * 128 : (ga + 1) * 128],
                        lo_q[0:64, jq * 512 : (jq + 1) * 512],
                        start=True,
                        stop=True,
                    )
                    nc.tensor.matmul(
                        sc[:, 512:1024],
                        hi_t[64:128, gb * 128 : (gb + 1) * 128],
                        hi_q[64:128, jq * 512 : (jq + 1) * 512],
                        start=True,
                        stop=True,
                    )
                    pt = WK.tile([128, 1024], bf16, tag="pts", name="pt")
                    nc.scalar.activation(pt[:], sc[:], EXP, scale=0.125)
                    pts.append(pt)
                    if m == 0:
                        if pending_final is not None:
                            pending_final()
                            pending_final = None
                        out_stage_copies()
                    emit_out_stage()
                    # interleave remaining projection work into the
                    # activation-bound steady state (after the exp emission so
                    # scores are never delayed behind projection work)
                    if bi == 0:
                        v_unit(m)
                        if m % 4 == 0 and m // 4 + 1 <= 7:
                            kt_unit(0, m // 4 + 1)
                        # pre-stage the next blocks' q projections so block
                        # transitions never burst 6 matmuls before scores
                        if m == 24:
                            enqueue_qt(*reversed(blocks[1]))
                        if m >= 24:
                            proj_step()
                    else:
                        if m == 0 and bi == 1:
                            for j2 in range(8):
                                enqueue_kt(2, j2)
                        if m == (12 if nm == 32 else 8) and bi + 1 < len(blocks):
                            njq, npr = blocks[bi + 1]
                            enqueue_qt(npr, njq)
                        if m == (20 if nm == 32 else 10) and bi + 2 < len(blocks):
                            njq, npr = blocks[bi + 2]
                            enqueue_qt(npr, njq)
                        proj_step()

                    # ctx runs one step behind exp so the PE overlaps the
                    # activation latency with the previous step's ctx
                    if m > 0:
                        emit_ctx(m - 1)
                # final step's ctx is deferred into the next block so
                # the transition never stalls on the last exp
                pending_final = (lambda f=emit_ctx, n=nm: f(n - 1))
                for hi, h in enumerate(heads):
                    out_stage_q.append((jq, h, cxs[hi], {"step": 0}))
            if pending_final is not None:
                pending_final()
                pending_final = None
            flush_out_stages()

    nc.compile()
    return nc


def _get_nc():
    if "nc" not in _CACHE:
        _CACHE["nc"] = _build()
    return _CACHE["nc"]


def _in_maps(hs, mask, Wq, bq, Wk, bk, Wv, bv):
    ident = np.eye(128, dtype=np.float32)
    maskt = np.ascontiguousarray(mask.reshape(NT, 128).T)  # [128, 32]
    hsT = np.ascontiguousarray(hs.astype(bf16np).T)  # [768, 4096] bf16
    hsqT = [
        np.ascontiguousarray(hs[sh * SQ : (sh + 1) * SQ, :].astype(bf16np).T)
        for sh in range(QS)
    ]

    def qk_chunks(W, hg):  # [768, :] f32 -> [128, 6*256] bf16: [h0|h1|h2|h2]
        out = np.zeros((128, NHC * WCC), bf16np)
        for c in range(NHC):
            blk = W[c * 128 : (c + 1) * 128, hg * CC : (hg + 1) * CC].astype(
                bf16np
            )
            out[:, c * WCC : c * WCC + CC] = blk
            out[:, c * WCC + CC : c * WCC + 256] = blk[:, 128:192]
        return out

    def v_chunks(W):  # augmented V weights -> [128, 6*195] bf16
        out = np.empty((128, NHC * VC), bf16np)
        for c in range(NHC):
            out[:, c * VC : (c + 1) * VC] = W[c * 128 : (c + 1) * 128, :].astype(
                bf16np
            )
        return out

    maps = []
    for core in range(N_CORES):
        hg, sh = core // QS, core % QS
        wv_aug = np.zeros((HID, VC), np.float32)
        bv_aug = np.zeros((1, VC), np.float32)
        for h in range(HPC):
            wv_aug[:, h * 65 : h * 65 + 64] = Wv[
                :, hg * CC + h * 64 : hg * CC + (h + 1) * 64
            ]
            bv_aug[0, h * 65 : h * 65 + 64] = bv[
                hg * CC + h * 64 : hg * CC + (h + 1) * 64
            ]
            bv_aug[0, h * 65 + 64] = 1.0
        # per-head bias columns on each head's partition half (h2 on both)
        bqt = np.zeros((128, HPC), np.float32)
        bkt = np.zeros((128, HPC), np.float32)
        for h, lo in ((0, 0), (1, 64)):
            bqt[lo : lo + 64, h] = bq[hg * CC + h * 64 : hg * CC + (h + 1) * 64]
            bkt[lo : lo + 64, h] = bk[hg * CC + h * 64 : hg * CC + (h + 1) * 64]
        for lo in (0, 64):
            bqt[lo : lo + 64, 2] = bq[hg * CC + 128 : hg * CC + 192]
            bkt[lo : lo + 64, 2] = bk[hg * CC + 128 : hg * CC + 192]
        maps.append(
            {
                "hsT": hsT,
                "hsqT": hsqT[sh],
                "wqb": qk_chunks(Wq, hg),
                "wkb": qk_chunks(Wk, hg),
                "wvb": v_chunks(wv_aug),
                "bqt": bqt,
                "bkt": bkt,
                "bvb": np.ascontiguousarray(
                    np.broadcast_to(bv_aug.astype(bf16np), (128, VC))
                ),
                "maskt": maskt,
                "ident": ident,
            }
        )
    return maps


def kernel(hidden_states, attention_mask, Wq, bq, Wk, bk, Wv, bv, **run_kwargs):
    hs = np.ascontiguousarray(np.asarray(hidden_states, np.float32).reshape(S, HID))
    mask = np.ascontiguousarray(np.asarray(attention_mask, np.float32).reshape(S))
    Wq = np.asarray(Wq, np.float32)
    Wk = np.asarray(Wk, np.float32)
    Wv = np.asarray(Wv, np.float32)
    bq = np.asarray(bq, np.float32)
    bk = np.asarray(bk, np.float32)
    bv = np.asarray(bv, np.float32)

    nc = _get_nc()
    maps = _in_maps(hs, mask, Wq, bq, Wk, bk, Wv, bv)
    res = bass_utils.run_bass_kernel_spmd(
        nc, maps, core_ids=list(range(N_CORES)), **run_kwargs
    )
    out = np.zeros((S, NH * HD), np.float32)
    for core in range(N_CORES):
        hg, sh = core // QS, core % QS
        out[sh * SQ : (sh + 1) * SQ, hg * CC : (hg + 1) * CC] = res.results[core][
            "out"
        ]
    if "trace" in run_kwargs:
        _CACHE["last_result"] = res
    return out.reshape(B, S, NH * HD)


# revision 29
# speedup vs baseline: 1.1797x; 1.1719x over previous
"""Trainium2 Bass kernel for BertSelfAttention (B=1, S=4096, HID=768, 12 heads).

Sharding: 8 cores = 4 head-groups x 2 query-halves. Each core computes 3 heads
for 2048 query rows against all 4096 keys, fused (scores never hit HBM).

Host-side sharding prep packs each core's inputs in their on-chip layout
(bf16, transposed hidden states, chunk-major weights), so the device spends no
time on layout transforms.

Per-core dataflow (bf16 matmuls, fp32 PSUM accumulation):
  - score matmuls contract only HD=64 partitions and run PAIRWISE CONCURRENT
    on the PE via row tiling: heads 0/2 hold Q^T/K^T on partitions 0:64,
    head 1 (and a duplicate copy of head 2) on partitions 64:128. Each
    gg step issues tile_position (0,0) and (64,0) matmuls that execute
    simultaneously on complementary halves of the PE array.
  - paired projection matmuls produce two heads per instruction (head 0 cols
    0:64 + head 1 cols 64:128 of the stationary weights); head 2's unit
    carries [h2|h2] so both partition halves get a copy.
  - scores land transposed (S^T[k, q]) in PSUM; one ScalarE Exp per
    [128, 1024] tile writes bf16 P^T straight to SBUF (scale=1/8 folded in).
  - additive attention mask handled exactly by scaling V rows (and the
    appended ones-column) with exp(mask[k]) computed on device.
  - V is augmented with a ones column per head, so the context matmul
    accumulates both sum(p*v) and sum(p) (the softmax denominator) in one
    PSUM group.
  - ctx^T [65, 512] tiles are PE-transposed back to [q, d] layout, divided by
    the denominator on VectorE, and DMA'd out.
  - projection/V work is hand-interleaved into the attention sweep so the PE
    fills activation bubbles instead of serializing up front.
"""

import sys

sys.path.insert(0, "/opt/trn_rl_repo")

import ml_dtypes
import numpy as np

import concourse.bacc as bacc
import concourse.mybir as mybir
import concourse.tile as tile
from concourse import bass_utils

B, S, HID = 1, 4096, 768
NH, HD = 12, 64
N_CORES = 8
HG = 4  # head-groups (tensor parallel)
QS = 2  # query splits (data parallel on sequence)
HPC = NH // HG  # 3 heads per core
SQ = S // QS  # 2048 query rows per core
CC = HPC * HD  # 192 projection columns per core
WCC = 256  # weight cols per chunk in wqb/wkb: [h0|h1|h2|h2]
VC = HPC * (HD + 1)  # 195 augmented V columns (ones col per head)
NHC = HID // 128  # 6 contraction chunks
NT = S // 128  # 32 key tiles

f32 = mybir.dt.float32
bf16 = mybir.dt.bfloat16
bf16np = ml_dtypes.bfloat16

_CACHE = {}


def _build():
    EXP = mybir.ActivationFunctionType.Exp
    nc = bacc.Bacc("TRN2", target_bir_lowering=False)

    hsT_d = nc.dram_tensor("hsT", [HID, S], bf16, kind="ExternalInput")
    hsqT_d = nc.dram_tensor("hsqT", [HID, SQ], bf16, kind="ExternalInput")
    wqb_d = nc.dram_tensor("wqb", [128, NHC * WCC], bf16, kind="ExternalInput")
    wkb_d = nc.dram_tensor("wkb", [128, NHC * WCC], bf16, kind="ExternalInput")
    wvb_d = nc.dram_tensor("wvb", [128, NHC * VC], bf16, kind="ExternalInput")
    bqt_d = nc.dram_tensor("bqt", [128, HPC], f32, kind="ExternalInput")
    bkt_d = nc.dram_tensor("bkt", [128, HPC], f32, kind="ExternalInput")
    bvb_d = nc.dram_tensor("bvb", [128, VC], bf16, kind="ExternalInput")
    maskt_d = nc.dram_tensor("maskt", [128, NT], f32, kind="ExternalInput")
    ident_d = nc.dram_tensor("ident", [128, 128], f32, kind="ExternalInput")
    out_d = nc.dram_tensor("out", [SQ, CC], f32, kind="ExternalOutput")

    with tile.TileContext(nc) as tc:
        with (
            tc.tile_pool(name="persist", bufs=1) as P,
            tc.tile_pool(name="work", bufs=6) as WK,
            tc.tile_pool(name="outp", bufs=2) as OP,
            tc.tile_pool(name="ppsum", bufs=2, space="PSUM") as PP,
            tc.tile_pool(name="bpsum", bufs=2, space="PSUM") as BP,
            tc.tile_pool(name="cpsum", bufs=2, space="PSUM") as CP,
        ):
            # ---- persistent SBUF tensors ----
            # chunk-major transposed activations: chunk c at cols [c*S, (c+1)*S)
            hsT = P.tile([128, NHC * S], bf16, tag="hsT")
            hsTq = P.tile([128, NHC * SQ], bf16, tag="hsTq")
            wqb = P.tile([128, NHC * WCC], bf16, tag="wqb")
            wkb = P.tile([128, NHC * WCC], bf16, tag="wkb")
            wvb = P.tile([128, NHC * VC], bf16, tag="wvb")
            bvb = P.tile([128, VC], bf16, tag="bvb")
            bqt = P.tile([128, HPC], f32, tag="bqt")
            bkt = P.tile([128, HPC], f32, tag="bkt")
            maskt = P.tile([128, NT], f32, tag="maskt")
            wmask = P.tile([128, NT], f32, tag="wmask")
            identf = P.tile([128, 128], f32, tag="identf")
            # qt/kt partition halves: qt[0] lo=h0, qt[1] hi=h1, qt[2] both=h2
            qt = [
                P.tile([128, SQ], bf16, tag=f"qt{h}", name=f"qt{h}")
                for h in range(HPC)
            ]
            kt = [
                P.tile([128, S], bf16, tag=f"kt{h}", name=f"kt{h}")
                for h in range(HPC)
            ]
            vv = P.tile([128, NT * VC], bf16, tag="vv")

            # ---- emission helpers ----
            hsT_3d = hsT.rearrange("p (c s) -> p c s", s=S)
            hsT_d3 = hsT_d.rearrange("(c p) s -> p c s", p=128)
            hsTq_3d = hsTq.rearrange("p (c s) -> p c s", s=SQ)
            hsqT_d3 = hsqT_d.rearrange("(c p) s -> p c s", p=128)

            def load_hsT_cols(s0, s1):
                nc.sync.dma_start(hsT_3d[:, :, s0:s1], hsT_d3[:, :, s0:s1])

            def load_hsqT_cols(s0, s1):
                nc.sync.dma_start(hsTq_3d[:, :, s0:s1], hsqT_d3[:, :, s0:s1])

            # projection units: the h0/h1 pair shares one matmul chain
            # (stationary cols 0:128 of the chunk), h2 uses cols 128:256
            # ([h2|h2] duplicated, so both halves get a copy)
            def proj_writeback(kind, key, ps):
                dst = qt if kind == "qt" else kt
                bias = bqt if kind == "qt" else bkt
                j = key[1]
                if key[0] == 0:
                    nc.vector.tensor_scalar_add(
                        dst[0][0:64, j * 512 : (j + 1) * 512],
                        ps[0:64, :],
                        bias[0:64, 0:1],
                    )
                    nc.vector.tensor_scalar_add(
                        dst[1][64:128, j * 512 : (j + 1) * 512],
                        ps[64:128, :],
                        bias[64:128, 1:2],
                    )
                else:
                    nc.vector.tensor_scalar_add(
                        dst[2][:, j * 512 : (j + 1) * 512],
                        ps[:],
                        bias[:, 2:3],
                    )

            qt_done = set()

            def qt_unit(hkey, j):
                key = (hkey, j)
                if key in qt_done:
                    return
                qt_done.add(key)
                coff = 0 if hkey == 0 else 128
                pq = PP.tile([128, 512], f32, tag="proj", name="pq")
                for c in range(NHC):
                    nc.tensor.matmul(
                        pq[:],
                        wqb[:, c * WCC + coff : c * WCC + coff + 128],
                        hsTq[:, c * SQ + j * 512 : c * SQ + (j + 1) * 512],
                        start=(c == 0),
                        stop=(c == NHC - 1),
                    )
                proj_writeback("qt", key, pq)

            kt_done = set()

            def kt_unit(hkey, j):
                # produces key block [512j, 512(j+1)) for the h0/h1 pair or h2
                key = (hkey, j)
                if key in kt_done:
                    return
                kt_done.add(key)
                coff = 0 if hkey == 0 else 128
                pk = PP.tile([128, 512], f32, tag="proj", name="pk")
                for c in range(NHC):
                    nc.tensor.matmul(
                        pk[:],
                        wkb[:, c * WCC + coff : c * WCC + coff + 128],
                        hsT[:, c * S + j * 512 : c * S + (j + 1) * 512],
                        start=(c == 0),
                        stop=(c == NHC - 1),
                    )
                proj_writeback("kt", key, pk)

            def v_unit(t):
                # V projection; bias add + mask scale on the (idle) VectorE
                pv = PP.tile([128, VC], f32, tag="proj", name="pv")
                for c in range(NHC):
                    nc.tensor.matmul(
                        pv[:],
                        hsT[:, c * S + t * 128 : c * S + (t + 1) * 128],
                        wvb[:, c * VC : (c + 1) * VC],
                        start=(c == 0),
                        stop=(c == NHC - 1),
                    )
                vt = WK.tile([128, VC], bf16, tag="vtmp", name="vt", bufs=2)
                nc.vector.tensor_tensor(vt[:], pv[:], bvb[:], mybir.AluOpType.add)
                nc.vector.tensor_scalar_mul(
                    vv[:, t * VC : (t + 1) * VC], vt[:], wmask[:, t : t + 1]
                )

            # ---- ramp: pipelined input loads + first-needed projections ----
            # mask load + exp first: the ScalarE is in-order, so this tiny
            # ACTIVATE must clear the queue before the first score exp; its
            # DMA must not sit behind the big activation transfers
            nc.sync.dma_start(maskt[:], maskt_d[:])
            nc.scalar.activation(wmask[:], maskt[:], EXP)
            load_hsqT_cols(0, 512)  # enough for qt(*, 0)
            nc.sync.dma_start(wqb[:], wqb_d[:])
            nc.sync.dma_start(bqt[:], bqt_d[:])
            load_hsT_cols(0, 512)  # enough for kt(0, 0)
            nc.sync.dma_start(wkb[:], wkb_d[:])
            nc.sync.dma_start(bkt[:], bkt_d[:])
            qt_unit(0, 0)
            kt_unit(0, 0)
            # V weights before the bulk hsT tail: v_unit(0) fires at the first
            # m-step and must not queue behind 2.3MB of activations
            nc.sync.dma_start(wvb[:], wvb_d[:])
            nc.sync.dma_start(bvb[:], bvb_d[:])
            nc.sync.dma_start(identf[:], ident_d[:])
            load_hsT_cols(512, 2048)
            load_hsT_cols(2048, 4096)
            load_hsqT_cols(512, SQ)

            # stepwise projection queues: one matmul per m-step so unit
            # bursts never overrun the per-step ScalarE slack
            qt_q = []
            kt_q = []

            def proj_step():
                q = qt_q if qt_q else kt_q
                if not q:
                    return
                st = q[0]
                c = st["step"]
                kind, key = st["kind"], st["key"]
                coff = 0 if key[0] == 0 else 128
                if c == 0:
                    st["ps"] = PP.tile([128, 512], f32, tag="proj", name="ps")
                ps = st["ps"]
                if kind == "qt":
                    nc.tensor.matmul(
                        ps[:],
                        wqb[:, c * WCC + coff : c * WCC + coff + 128],
                        hsTq[:, c * SQ + key[1] * 512 : c * SQ + (key[1] + 1) * 512],
                        start=(c == 0),
                        stop=(c == NHC - 1),
                    )
                else:
                    nc.tensor.matmul(
                        ps[:],
                        wkb[:, c * WCC + coff : c * WCC + coff + 128],
                        hsT[:, c * S + key[1] * 512 : c * S + (key[1] + 1) * 512],
                        start=(c == 0),
                        stop=(c == NHC - 1),
                    )
                if c == NHC - 1:
                    proj_writeback(kind, key, ps)
                    q.pop(0)
                    return
                st["step"] += 1

            def enqueue_qt(hkey, j):
                key = (hkey, j)
                if key in qt_done:
                    return
                qt_done.add(key)
                qt_q.append({"kind": "qt", "key": key, "step": 0})

            def enqueue_kt(hkey, j):
                key = (hkey, j)
                if key in kt_done:
                    return
                kt_done.add(key)
                kt_q.append({"kind": "kt", "key": key, "step": 0})

            # deferred out-stage, pipelined into the next block's m-loop
            out_stage_q = []

            def out_stage_copies():
                # front-load the PSUM-freeing cs copies for every queued
                # entry so the next block's ctx chains (which reuse the CP
                # banks) depend on already-emitted DVE work
                for ent in out_stage_q:
                    st = ent[3]
                    if st["step"] == 0:
                        cs = OP.tile([65, 512], f32, tag="cs", name="cs")
                        nc.vector.tensor_copy(cs[:], ent[2][:])
                        st["cs"] = cs
                        st["ot"] = OP.tile([128, 4 * 64], f32, tag="ot", name="ot")
                        st["step"] = 1

            def emit_out_stage():
                if not out_stage_q:
                    return
                jq, h, cx, st = out_stage_q[0]
                if st["step"] == 0:
                    cs = OP.tile([65, 512], f32, tag="cs", name="cs")
                    nc.vector.tensor_copy(cs[:], cx[:])
                    st["cs"] = cs
                    st["ot"] = OP.tile([128, 4 * 64], f32, tag="ot", name="ot")
                elif st["step"] == 1:
                    # all four transposes back-to-back
                    cs = st["cs"]
                    tp2 = PP.tile([128, 4 * 65], f32, tag="proj", name="tp2")
                    st["tp2"] = tp2
                    for t4 in range(4):
                        nc.tensor.transpose(
                            tp2[:, t4 * 65 : (t4 + 1) * 65],
                            cs[:, t4 * 128 : (t4 + 1) * 128],
                            identf[0:65, 0:65],
                        )
                elif st["step"] <= 5:
                    t4 = st["step"] - 2
                    tp2, ot = st["tp2"], st["ot"]
                    if t4 == 0:
                        # one batched reciprocal over the four denominator
                        # columns (strided view) instead of four tiny ones
                        rc = OP.tile([128, 4], f32, tag="rc", name="rc")
                        st["rc"] = rc
                        nc.vector.reciprocal(
                            rc[:],
                            tp2.rearrange("p (t c) -> p t c", c=65)[:, :, 64],
                        )
                    rc = st["rc"]
                    nc.vector.tensor_scalar_mul(
                        ot[:, t4 * 64 : (t4 + 1) * 64],
                        tp2[:, t4 * 65 : t4 * 65 + 64],
                        rc[:, t4 : t4 + 1],
                    )
                    # per-chunk DMA: spreads the writeback across queues and
                    # shrinks the post-compute tail to one 32KB transfer
                    nc.sync.dma_start(
                        out_d[
                            jq * 512 + t4 * 128 : jq * 512 + (t4 + 1) * 128,
                            h * 64 : (h + 1) * 64,
                        ],
                        ot[:, t4 * 64 : (t4 + 1) * 64],
                    )
                    if t4 == 3:
                        out_stage_q.pop(0)
                        return
                st["step"] += 1

            def flush_out_stages():
                while out_stage_q:
                    emit_out_stage()

            # ---- attention sweep over head-pair blocks ----
            # pr=0: heads (0,1) paired across partition halves, 32 m-steps
            #       (one key tile per head per step)
            # pr=2: head 2 paired with its own duplicate, 16 m-steps
            #       (key tiles 2m / 2m+1)
            blocks = [(jq, pr) for pr in (0, 2) for jq in range(SQ // 512)]
            pending_final = None

            for bi, (jq, pr) in enumerate(blocks):
                qt_unit(pr, jq)
                nm = 32 if pr == 0 else 16
                if pr == 0:
                    cxs = [
                        CP.tile([65, 512], f32, tag="ctx", name=f"cx{bi}_0"),
                        CP.tile([65, 512], f32, tag="ctx", name=f"cx{bi}_1"),
                    ]
                    heads = (0, 1)
                else:
                    cxs = [CP.tile([65, 512], f32, tag="ctx", name=f"cx{bi}_2")]
                    heads = (2,)
                pts = []

                def emit_ctx(pm, cxs=cxs, pts=pts, pr=pr, nm=nm):
                    pt = pts[pm]
                    if pr == 0:
                        for hi, h in enumerate((0, 1)):
                            nc.tensor.matmul(
                                cxs[hi][:],
                                vv[:, pm * VC + h * 65 : pm * VC + h * 65 + 65],
                                pt[:, hi * 512 : (hi + 1) * 512],
                                start=(pm == 0),
                                stop=(pm == nm - 1),
                            )
                    else:
                        for gi, g in enumerate((2 * pm, 2 * pm + 1)):
                            nc.tensor.matmul(
                                cxs[0][:],
                                vv[:, g * VC + 2 * 65 : g * VC + 2 * 65 + 65],
                                pt[:, gi * 512 : (gi + 1) * 512],
                                start=(g == 0),
                                stop=(g == NT - 1),
                            )

                for m in range(nm):
                    # paired score matmuls: tile (0,0) on partitions 0:64 and
                    # tile (64,0) on partitions 64:128 run concurrently
                    sc = BP.tile([128, 1024], f32, tag="big", name="sc")
                    if pr == 0:
                        ga, gb = m, m
                        lo_t, hi_t = kt[0], kt[1]
                        lo_q, hi_q = qt[0], qt[1]
                    else:
                        ga, gb = 2 * m, 2 * m + 1
                        lo_t = hi_t = kt[2]
                        lo_q = hi_q = qt[2]
                    nc.tensor.matmul(
                        sc[:, 0:512],
                        lo_t[0:64, ga * 128 : (ga + 1) * 128],
                        lo_q[0:64, jq * 512 : (jq + 1) * 512],
                        start=True,
                        stop=True,
                    )
                    nc.tensor.matmul(
                        sc[:, 512:1024],
                        hi_t[64:128, gb * 128 : (gb + 1) * 128],
                        hi_q[64:128, jq * 512 : (jq + 1) * 512],
                        start=True,
                        stop=True,
                    )
                    pt = WK.tile([128, 1024], bf16, tag="pts", name="pt")
                    nc.scalar.activation(pt[:], sc[:], EXP, scale=0.125)
                    pts.append(pt)
                    if m == 0:
                        if pending_final is not None:
                            pending_final()
                            pending_final = None
                        out_stage_copies()
                    emit_out_stage()
                    # interleave remaining projection work into the
                    # activation-bound steady state (after the exp emission so
                    # scores are never delayed behind projection work)
                    if bi == 0:
                        v_unit(m)
                        if m % 4 == 0 and m // 4 + 1 <= 7:
                            kt_unit(0, m // 4 + 1)
                        # pre-stage the next blocks' q projections so block
                        # transitions never burst 6 matmuls before scores
                        if m == 24:
                            enqueue_qt(*reversed(blocks[1]))
                        if m >= 24:
                            proj_step()
                    else:
                        if m == 0 and bi == 1:
                            for j2 in range(8):
                                enqueue_kt(2, j2)
                        if m == (12 if nm == 32 else 8) and bi + 1 < len(blocks):
                            njq, npr = blocks[bi + 1]
                            enqueue_qt(npr, njq)
                        if m == (20 if nm == 32 else 10) and bi + 2 < len(blocks):
                            njq, npr = blocks[bi + 2]
                            enqueue_qt(npr, njq)
                        proj_step()

                    # ctx runs one step behind exp so the PE overlaps the
                    # activation latency with the previous step's ctx
                    if m > 0:
                        emit_ctx(m - 1)
                # final step's ctx is deferred into the next block so
                # the transition never stalls on the last exp
                pending_final = (lambda f=emit_ctx, n=nm: f(n - 1))
                for hi, h in enumerate(heads):
                    out_stage_q.append((jq, h, cxs[hi], {"step": 0}))
            if pending_final is not None:
                pending_final()
                pending_final = None
            flush_out_stages()

    nc.compile()
    return nc


def _get_nc():
    if "nc" not in _CACHE:
        _CACHE["nc"] = _build()
    return _CACHE["nc"]


def _in_maps(hs, mask, Wq, bq, Wk, bk, Wv, bv):
    ident = np.eye(128, dtype=np.float32)
    maskt = np.ascontiguousarray(mask.reshape(NT, 128).T)  # [128, 32]
    hsT = np.ascontiguousarray(hs.astype(bf16np).T)  # [768, 4096] bf16
    hsqT = [
        np.ascontiguousarray(hs[sh * SQ : (sh + 1) * SQ, :].astype(bf16np).T)
        for sh in range(QS)
    ]

    def qk_chunks(W, hg):  # [768, :] f32 -> [128, 6*256] bf16: [h0|h1|h2|h2]
        out = np.zeros((128, NHC * WCC), bf16np)
        for c in range(NHC):
            blk = W[c * 128 : (c + 1) * 128, hg * CC : (hg + 1) * CC].astype(
                bf16np
            )
            out[:, c * WCC : c * WCC + CC] = blk
            out[:, c * WCC + CC : c * WCC + 256] = blk[:, 128:192]
        return out

    def v_chunks(W):  # augmented V weights -> [128, 6*195] bf16
        out = np.empty((128, NHC * VC), bf16np)
        for c in range(NHC):
            out[:, c * VC : (c + 1) * VC] = W[c * 128 : (c + 1) * 128, :].astype(
                bf16np
            )
        return out

    maps = []
    for core in range(N_CORES):
        hg, sh = core // QS, core % QS
        wv_aug = np.zeros((HID, VC), np.float32)
        bv_aug = np.zeros((1, VC), np.float32)
        for h in range(HPC):
            wv_aug[:, h * 65 : h * 65 + 64] = Wv[
                :, hg * CC + h * 64 : hg * CC + (h + 1) * 64
            ]
            bv_aug[0, h * 65 : h * 65 + 64] = bv[
                hg * CC + h * 64 : hg * CC + (h + 1) * 64
            ]
            bv_aug[0, h * 65 + 64] = 1.0
        # per-head bias columns on each head's partition half (h2 on both)
        bqt = np.zeros((128, HPC), np.float32)
        bkt = np.zeros((128, HPC), np.float32)
        for h, lo in ((0, 0), (1, 64)):
            bqt[lo : lo + 64, h] = bq[hg * CC + h * 64 : hg * CC + (h + 1) * 64]
            bkt[lo : lo + 64, h] = bk[hg * CC + h * 64 : hg * CC + (h + 1) * 64]
        for lo in (0, 64):
            bqt[lo : lo + 64, 2] = bq[hg * CC + 128 : hg * CC + 192]
            bkt[lo : lo + 64, 2] = bk[hg * CC + 128 : hg * CC + 192]
        maps.append(
            {
                "hsT": hsT,
                "hsqT": hsqT[sh],
                "wqb": qk_chunks(Wq, hg),
                "wkb": qk_chunks(Wk, hg),
                "wvb": v_chunks(wv_aug),
                "bqt": bqt,
                "bkt": bkt,
                "bvb": np.ascontiguousarray(
                    np.broadcast_to(bv_aug.astype(bf16np), (128, VC))
                ),
                "maskt": maskt,
                "ident": ident,
            }
        )
    return maps


def kernel(hidden_states, attention_mask, Wq, bq, Wk, bk, Wv, bv, **run_kwargs):
    hs = np.ascontiguousarray(np.asarray(hidden_states, np.float32).reshape(S, HID))
    mask = np.ascontiguousarray(np.asarray(attention_mask, np.float32).reshape(S))
    Wq = np.asarray(Wq, np.float32)
    Wk = np.asarray(Wk, np.float32)
    Wv = np.asarray(Wv, np.float32)
    bq = np.asarray(bq, np.float32)
    bk = np.asarray(bk, np.float32)
    bv = np.asarray(bv, np.float32)

    nc = _get_nc()
    maps = _in_maps(hs, mask, Wq, bq, Wk, bk, Wv, bv)
    res = bass_utils.run_bass_kernel_spmd(
        nc, maps, core_ids=list(range(N_CORES)), **run_kwargs
    )
    out = np.zeros((S, NH * HD), np.float32)
    for core in range(N_CORES):
        hg, sh = core // QS, core % QS
        out[sh * SQ : (sh + 1) * SQ, hg * CC : (hg + 1) * CC] = res.results[core][
            "out"
        ]
    if "trace" in run_kwargs:
        _CACHE["last_result"] = res
    return out.reshape(B, S, NH * HD)


# revision 30
# speedup vs baseline: 1.1949x; 1.0129x over previous
"""Trainium2 Bass kernel for BertSelfAttention (B=1, S=4096, HID=768, 12 heads).

Sharding: 8 cores = 4 head-groups x 2 query-halves. Each core computes 3 heads
for 2048 query rows against all 4096 keys, fused (scores never hit HBM).

Host-side sharding prep packs each core's inputs in their on-chip layout
(bf16, transposed hidden states, chunk-major weights), so the device spends no
time on layout transforms.

Per-core dataflow (bf16 matmuls, fp32 PSUM accumulation):
  - score matmuls contract only HD=64 partitions and run PAIRWISE CONCURRENT
    on the PE via row tiling: heads 0/2 hold Q^T/K^T on partitions 0:64,
    head 1 (and a duplicate copy of head 2) on partitions 64:128. Each
    gg step issues tile_position (0,0) and (64,0) matmuls that execute
    simultaneously on complementary halves of the PE array.
  - paired projection matmuls produce two heads per instruction (head 0 cols
    0:64 + head 1 cols 64:128 of the stationary weights); head 2's unit
    carries [h2|h2] so both partition halves get a copy.
  - scores land transposed (S^T[k, q]) in PSUM; one ScalarE Exp per
    [128, 1024] tile writes bf16 P^T straight to SBUF (scale=1/8 folded in).
  - additive attention mask handled exactly by scaling V rows (and the
    appended ones-column) with exp(mask[k]) computed on device.
  - V is augmented with a ones column per head, so the context matmul
    accumulates both sum(p*v) and sum(p) (the softmax denominator) in one
    PSUM group.
  - ctx^T [65, 512] tiles are PE-transposed back to [q, d] layout, divided by
    the denominator on VectorE, and DMA'd out.
  - projection/V work is hand-interleaved into the attention sweep so the PE
    fills activation bubbles instead of serializing up front.
"""

import sys

sys.path.insert(0, "/opt/trn_rl_repo")

import ml_dtypes
import numpy as np

import concourse.bacc as bacc
import concourse.mybir as mybir
import concourse.tile as tile
from concourse import bass_utils

B, S, HID = 1, 4096, 768
NH, HD = 12, 64
N_CORES = 8
HG = 4  # head-groups (tensor parallel)
QS = 2  # query splits (data parallel on sequence)
HPC = NH // HG  # 3 heads per core
SQ = S // QS  # 2048 query rows per core
CC = HPC * HD  # 192 projection columns per core
WCC = 256  # weight cols per chunk in wqb/wkb: [h0|h1|h2|h2]
VC = HPC * (HD + 1)  # 195 augmented V columns (ones col per head)
NHC = HID // 128  # 6 contraction chunks
NT = S // 128  # 32 key tiles

f32 = mybir.dt.float32
bf16 = mybir.dt.bfloat16
bf16np = ml_dtypes.bfloat16

_CACHE = {}


def _build():
    EXP = mybir.ActivationFunctionType.Exp
    nc = bacc.Bacc("TRN2", target_bir_lowering=False)

    hsT_d = nc.dram_tensor("hsT", [HID, S], bf16, kind="ExternalInput")
    hsqT_d = nc.dram_tensor("hsqT", [HID, SQ], bf16, kind="ExternalInput")
    wqb_d = nc.dram_tensor("wqb", [128, NHC * WCC], bf16, kind="ExternalInput")
    wkb_d = nc.dram_tensor("wkb", [128, NHC * WCC], bf16, kind="ExternalInput")
    wvb_d = nc.dram_tensor("wvb", [128, NHC * VC], bf16, kind="ExternalInput")
    bqt_d = nc.dram_tensor("bqt", [128, HPC], f32, kind="ExternalInput")
    bkt_d = nc.dram_tensor("bkt", [128, HPC], f32, kind="ExternalInput")
    bvb_d = nc.dram_tensor("bvb", [128, VC], bf16, kind="ExternalInput")
    maskt_d = nc.dram_tensor("maskt", [128, NT], f32, kind="ExternalInput")
    ident_d = nc.dram_tensor("ident", [128, 128], f32, kind="ExternalInput")
    out_d = nc.dram_tensor("out", [SQ, CC], f32, kind="ExternalOutput")

    with tile.TileContext(nc) as tc:
        with (
            tc.tile_pool(name="persist", bufs=1) as P,
            tc.tile_pool(name="work", bufs=6) as WK,
            tc.tile_pool(name="outp", bufs=2) as OP,
            tc.tile_pool(name="ppsum", bufs=2, space="PSUM") as PP,
            tc.tile_pool(name="bpsum", bufs=2, space="PSUM") as BP,
            tc.tile_pool(name="cpsum", bufs=2, space="PSUM") as CP,
        ):
            # ---- persistent SBUF tensors ----
            # chunk-major transposed activations: chunk c at cols [c*S, (c+1)*S)
            hsT = P.tile([128, NHC * S], bf16, tag="hsT")
            hsTq = P.tile([128, NHC * SQ], bf16, tag="hsTq")
            wqb = P.tile([128, NHC * WCC], bf16, tag="wqb")
            wkb = P.tile([128, NHC * WCC], bf16, tag="wkb")
            wvb = P.tile([128, NHC * VC], bf16, tag="wvb")
            bvb = P.tile([128, VC], bf16, tag="bvb")
            bqt = P.tile([128, HPC], f32, tag="bqt")
            bkt = P.tile([128, HPC], f32, tag="bkt")
            maskt = P.tile([128, NT], f32, tag="maskt")
            wmask = P.tile([128, NT], f32, tag="wmask")
            identf = P.tile([128, 128], f32, tag="identf")
            # qt/kt partition halves: qt[0] lo=h0, qt[1] hi=h1, qt[2] both=h2
            qt = [
                P.tile([128, SQ], bf16, tag=f"qt{h}", name=f"qt{h}")
                for h in range(HPC)
            ]
            kt = [
                P.tile([128, S], bf16, tag=f"kt{h}", name=f"kt{h}")
                for h in range(HPC)
            ]
            vv = P.tile([128, NT * VC], bf16, tag="vv")

            # ---- emission helpers ----
            hsT_3d = hsT.rearrange("p (c s) -> p c s", s=S)
            hsT_d3 = hsT_d.rearrange("(c p) s -> p c s", p=128)
            hsTq_3d = hsTq.rearrange("p (c s) -> p c s", s=SQ)
            hsqT_d3 = hsqT_d.rearrange("(c p) s -> p c s", p=128)

            def load_hsT_cols(s0, s1):
                nc.sync.dma_start(hsT_3d[:, :, s0:s1], hsT_d3[:, :, s0:s1])

            def load_hsqT_cols(s0, s1):
                nc.sync.dma_start(hsTq_3d[:, :, s0:s1], hsqT_d3[:, :, s0:s1])

            # projection units: the h0/h1 pair shares one matmul chain
            # (stationary cols 0:128 of the chunk), h2 uses cols 128:256
            # ([h2|h2] duplicated, so both halves get a copy)
            def proj_writeback(kind, key, ps):
                dst = qt if kind == "qt" else kt
                bias = bqt if kind == "qt" else bkt
                j = key[1]
                if key[0] == 0:
                    nc.vector.tensor_scalar_add(
                        dst[0][0:64, j * 512 : (j + 1) * 512],
                        ps[0:64, :],
                        bias[0:64, 0:1],
                    )
                    nc.vector.tensor_scalar_add(
                        dst[1][64:128, j * 512 : (j + 1) * 512],
                        ps[64:128, :],
                        bias[64:128, 1:2],
                    )
                else:
                    nc.vector.tensor_scalar_add(
                        dst[2][:, j * 512 : (j + 1) * 512],
                        ps[:],
                        bias[:, 2:3],
                    )

            qt_done = set()

            def qt_unit(hkey, j):
                key = (hkey, j)
                if key in qt_done:
                    return
                qt_done.add(key)
                coff = 0 if hkey == 0 else 128
                pq = PP.tile([128, 512], f32, tag="proj", name="pq")
                for c in range(NHC):
                    nc.tensor.matmul(
                        pq[:],
                        wqb[:, c * WCC + coff : c * WCC + coff + 128],
                        hsTq[:, c * SQ + j * 512 : c * SQ + (j + 1) * 512],
                        start=(c == 0),
                        stop=(c == NHC - 1),
                    )
                proj_writeback("qt", key, pq)

            kt_done = set()

            def kt_unit(hkey, j):
                # produces key block [512j, 512(j+1)) for the h0/h1 pair or h2
                key = (hkey, j)
                if key in kt_done:
                    return
                kt_done.add(key)
                coff = 0 if hkey == 0 else 128
                pk = PP.tile([128, 512], f32, tag="proj", name="pk")
                for c in range(NHC):
                    nc.tensor.matmul(
                        pk[:],
                        wkb[:, c * WCC + coff : c * WCC + coff + 128],
                        hsT[:, c * S + j * 512 : c * S + (j + 1) * 512],
                        start=(c == 0),
                        stop=(c == NHC - 1),
                    )
                proj_writeback("kt", key, pk)

            def v_unit(t):
                # V projection; bias add + mask scale on the (idle) VectorE
                pv = PP.tile([128, VC], f32, tag="proj", name="pv")
                for c in range(NHC):
                    nc.tensor.matmul(
                        pv[:],
                        hsT[:, c * S + t * 128 : c * S + (t + 1) * 128],
                        wvb[:, c * VC : (c + 1) * VC],
                        start=(c == 0),
                        stop=(c == NHC - 1),
                    )
                vt = WK.tile([128, VC], bf16, tag="vtmp", name="vt", bufs=2)
                nc.vector.tensor_tensor(vt[:], pv[:], bvb[:], mybir.AluOpType.add)
                nc.vector.tensor_scalar_mul(
                    vv[:, t * VC : (t + 1) * VC], vt[:], wmask[:, t : t + 1]
                )

            # ---- ramp: pipelined input loads + first-needed projections ----
            # mask load + exp first: the ScalarE is in-order, so this tiny
            # ACTIVATE must clear the queue before the first score exp; its
            # DMA must not sit behind the big activation transfers
            nc.sync.dma_start(maskt[:], maskt_d[:])
            nc.scalar.activation(wmask[:], maskt[:], EXP)
            load_hsqT_cols(0, 512)  # enough for qt(*, 0)
            nc.sync.dma_start(wqb[:], wqb_d[:])
            nc.sync.dma_start(bqt[:], bqt_d[:])
            load_hsT_cols(0, 512)  # enough for kt(0, 0)
            nc.sync.dma_start(wkb[:], wkb_d[:])
            nc.sync.dma_start(bkt[:], bkt_d[:])
            qt_unit(0, 0)
            kt_unit(0, 0)
            load_hsT_cols(512, 2048)
            nc.sync.dma_start(wvb[:], wvb_d[:])
            nc.sync.dma_start(bvb[:], bvb_d[:])
            nc.sync.dma_start(identf[:], ident_d[:])
            load_hsT_cols(2048, 4096)
            load_hsqT_cols(512, SQ)

            # stepwise projection queues: one matmul per m-step so unit
            # bursts never overrun the per-step ScalarE slack
            qt_q = []
            kt_q = []

            def proj_step():
                q = qt_q if qt_q else kt_q
                if not q:
                    return
                st = q[0]
                c = st["step"]
                kind, key = st["kind"], st["key"]
                coff = 0 if key[0] == 0 else 128
                if c == 0:
                    st["ps"] = PP.tile([128, 512], f32, tag="proj", name="ps")
                ps = st["ps"]
                if kind == "qt":
                    nc.tensor.matmul(
                        ps[:],
                        wqb[:, c * WCC + coff : c * WCC + coff + 128],
                        hsTq[:, c * SQ + key[1] * 512 : c * SQ + (key[1] + 1) * 512],
                        start=(c == 0),
                        stop=(c == NHC - 1),
                    )
                else:
                    nc.tensor.matmul(
                        ps[:],
                        wkb[:, c * WCC + coff : c * WCC + coff + 128],
                        hsT[:, c * S + key[1] * 512 : c * S + (key[1] + 1) * 512],
                        start=(c == 0),
                        stop=(c == NHC - 1),
                    )
                if c == NHC - 1:
                    proj_writeback(kind, key, ps)
                    q.pop(0)
                    return
                st["step"] += 1

            def enqueue_qt(hkey, j):
                key = (hkey, j)
                if key in qt_done:
                    return
                qt_done.add(key)
                qt_q.append({"kind": "qt", "key": key, "step": 0})

            def enqueue_kt(hkey, j):
                key = (hkey, j)
                if key in kt_done:
                    return
                kt_done.add(key)
                kt_q.append({"kind": "kt", "key": key, "step": 0})

            # deferred out-stage, pipelined into the next block's m-loop
            out_stage_q = []

            def out_stage_copies():
                # front-load the PSUM-freeing cs copies for every queued
                # entry so the next block's ctx chains (which reuse the CP
                # banks) depend on already-emitted DVE work
                for ent in out_stage_q:
                    st = ent[3]
                    if st["step"] == 0:
                        cs = OP.tile([65, 512], f32, tag="cs", name="cs")
                        nc.vector.tensor_copy(cs[:], ent[2][:])
                        st["cs"] = cs
                        st["ot"] = OP.tile([128, 4 * 64], f32, tag="ot", name="ot")
                        st["step"] = 1

            def emit_out_stage():
                if not out_stage_q:
                    return
                jq, h, cx, st = out_stage_q[0]
                if st["step"] == 0:
                    cs = OP.tile([65, 512], f32, tag="cs", name="cs")
                    nc.vector.tensor_copy(cs[:], cx[:])
                    st["cs"] = cs
                    st["ot"] = OP.tile([128, 4 * 64], f32, tag="ot", name="ot")
                elif st["step"] == 1:
                    # all four transposes back-to-back
                    cs = st["cs"]
                    tp2 = PP.tile([128, 4 * 65], f32, tag="proj", name="tp2")
                    st["tp2"] = tp2
                    for t4 in range(4):
                        nc.tensor.transpose(
                            tp2[:, t4 * 65 : (t4 + 1) * 65],
                            cs[:, t4 * 128 : (t4 + 1) * 128],
                            identf[0:65, 0:65],
                        )
                elif st["step"] <= 5:
                    t4 = st["step"] - 2
                    tp2, ot = st["tp2"], st["ot"]
                    rc = OP.tile([128, 1], f32, tag="rc", name="rc")
                    nc.vector.reciprocal(rc[:], tp2[:, t4 * 65 + 64 : t4 * 65 + 65])
                    nc.vector.tensor_scalar_mul(
                        ot[:, t4 * 64 : (t4 + 1) * 64],
                        tp2[:, t4 * 65 : t4 * 65 + 64],
                        rc[:],
                    )
                    # per-chunk DMA: spreads the writeback across queues and
                    # shrinks the post-compute tail to one 32KB transfer
                    nc.sync.dma_start(
                        out_d[
                            jq * 512 + t4 * 128 : jq * 512 + (t4 + 1) * 128,
                            h * 64 : (h + 1) * 64,
                        ],
                        ot[:, t4 * 64 : (t4 + 1) * 64],
                    )
                    if t4 == 3:
                        out_stage_q.pop(0)
                        return
                st["step"] += 1

            def flush_out_stages():
                while out_stage_q:
                    emit_out_stage()

            # ---- attention sweep over head-pair blocks ----
            # pr=0: heads (0,1) paired across partition halves, 32 m-steps
            #       (one key tile per head per step)
            # pr=2: head 2 paired with its own duplicate, 16 m-steps
            #       (key tiles 2m / 2m+1)
            blocks = [(jq, pr) for pr in (0, 2) for jq in range(SQ // 512)]
            pending_final = None

            for bi, (jq, pr) in enumerate(blocks):
                qt_unit(pr, jq)
                nm = 32 if pr == 0 else 16
                if pr == 0:
                    cxs = [
                        CP.tile([65, 512], f32, tag="ctx", name=f"cx{bi}_0"),
                        CP.tile([65, 512], f32, tag="ctx", name=f"cx{bi}_1"),
                    ]
                    heads = (0, 1)
                else:
                    cxs = [CP.tile([65, 512], f32, tag="ctx", name=f"cx{bi}_2")]
                    heads = (2,)
                pts = []

                def emit_ctx(pm, cxs=cxs, pts=pts, pr=pr, nm=nm):
                    pt = pts[pm]
                    if pr == 0:
                        for hi, h in enumerate((0, 1)):
                            nc.tensor.matmul(
                                cxs[hi][:],
                                vv[:, pm * VC + h * 65 : pm * VC + h * 65 + 65],
                                pt[:, hi * 512 : (hi + 1) * 512],
                                start=(pm == 0),
                                stop=(pm == nm - 1),
                            )
                    else:
                        for gi, g in enumerate((2 * pm, 2 * pm + 1)):
                            nc.tensor.matmul(
                                cxs[0][:],
                                vv[:, g * VC + 2 * 65 : g * VC + 2 * 65 + 65],
                                pt[:, gi * 512 : (gi + 1) * 512],
                                start=(g == 0),
                                stop=(g == NT - 1),
                            )

                for m in range(nm):
                    # paired score matmuls: tile (0,0) on partitions 0:64 and
                    # tile (64,0) on partitions 64:128 run concurrently
                    sc = BP.tile([128, 1024], f32, tag="big", name="sc")
                    if pr == 0:
                        ga, gb = m, m
                        lo_t, hi_t = kt[0], kt[1]
                        lo_q, hi_q = qt[0], qt[1]
                    else:
                        ga, gb = 2 * m, 2 * m + 1
                        lo_t = hi_t = kt[2]
                        lo_q = hi_q = qt[2]
                    nc.tensor.matmul(
                        sc[:, 0:512],
                        lo_t[0:64, ga * 128 : (ga + 1) * 128],
                        lo_q[0:64, jq * 512 : (jq + 1) * 512],
                        start=True,
                        stop=True,
                    )
                    nc.tensor.matmul(
                        sc[:, 512:1024],
                        hi_t[64:128, gb * 128 : (gb + 1) * 128],
                        hi_q[64:128, jq * 512 : (jq + 1) * 512],
                        start=True,
                        stop=True,
                    )
                    pt = WK.tile([128, 1024], bf16, tag="pts", name="pt")
                    nc.scalar.activation(pt[:], sc[:], EXP, scale=0.125)
                    pts.append(pt)
                    if m == 0:
                        if pending_final is not None:
                            pending_final()
                            pending_final = None
                        out_stage_copies()
                    emit_out_stage()
                    # interleave remaining projection work into the
                    # activation-bound steady state (after the exp emission so
                    # scores are never delayed behind projection work)
                    if bi == 0:
                        v_unit(m)
                        if m % 4 == 0 and m // 4 + 1 <= 7:
                            kt_unit(0, m // 4 + 1)
                        # pre-stage the next blocks' q projections so block
                        # transitions never burst 6 matmuls before scores
                        if m == 24:
                            enqueue_qt(*reversed(blocks[1]))
                        if m >= 24:
                            proj_step()
                    else:
                        if m == 0 and bi == 1:
                            for j2 in range(8):
                                enqueue_kt(2, j2)
                        if m == (12 if nm == 32 else 8) and bi + 1 < len(blocks):
                            njq, npr = blocks[bi + 1]
                            enqueue_qt(npr, njq)
                        if m == (20 if nm == 32 else 10) and bi + 2 < len(blocks):
                            njq, npr = blocks[bi + 2]
                            enqueue_qt(npr, njq)
                        proj_step()

                    # ctx runs one step behind exp so the PE overlaps the
                    # activation latency with the previous step's ctx
                    if m > 0:
                        emit_ctx(m - 1)
                # final step's ctx is deferred into the next block so
                # the transition never stalls on the last exp
                pending_final = (lambda f=emit_ctx, n=nm: f(n - 1))
                for hi, h in enumerate(heads):
                    out_stage_q.append((jq, h, cxs[hi], {"step": 0}))
            if pending_final is not None:
                pending_final()
                pending_final = None
            flush_out_stages()

    nc.compile()
    return nc


def _get_nc():
    if "nc" not in _CACHE:
        _CACHE["nc"] = _build()
    return _CACHE["nc"]


def _in_maps(hs, mask, Wq, bq, Wk, bk, Wv, bv):
    ident = np.eye(128, dtype=np.float32)
    maskt = np.ascontiguousarray(mask.reshape(NT, 128).T)  # [128, 32]
    hsT = np.ascontiguousarray(hs.astype(bf16np).T)  # [768, 4096] bf16
    hsqT = [
        np.ascontiguousarray(hs[sh * SQ : (sh + 1) * SQ, :].astype(bf16np).T)
        for sh in range(QS)
    ]

    def qk_chunks(W, hg):  # [768, :] f32 -> [128, 6*256] bf16: [h0|h1|h2|h2]
        out = np.zeros((128, NHC * WCC), bf16np)
        for c in range(NHC):
            blk = W[c * 128 : (c + 1) * 128, hg * CC : (hg + 1) * CC].astype(
                bf16np
            )
            out[:, c * WCC : c * WCC + CC] = blk
            out[:, c * WCC + CC : c * WCC + 256] = blk[:, 128:192]
        return out

    def v_chunks(W):  # augmented V weights -> [128, 6*195] bf16
        out = np.empty((128, NHC * VC), bf16np)
        for c in range(NHC):
            out[:, c * VC : (c + 1) * VC] = W[c * 128 : (c + 1) * 128, :].astype(
                bf16np
            )
        return out

    maps = []
    for core in range(N_CORES):
        hg, sh = core // QS, core % QS
        wv_aug = np.zeros((HID, VC), np.float32)
        bv_aug = np.zeros((1, VC), np.float32)
        for h in range(HPC):
            wv_aug[:, h * 65 : h * 65 + 64] = Wv[
                :, hg * CC + h * 64 : hg * CC + (h + 1) * 64
            ]
            bv_aug[0, h * 65 : h * 65 + 64] = bv[
                hg * CC + h * 64 : hg * CC + (h + 1) * 64
            ]
            bv_aug[0, h * 65 + 64] = 1.0
        # per-head bias columns on each head's partition half (h2 on both)
        bqt = np.zeros((128, HPC), np.float32)
        bkt = np.zeros((128, HPC), np.float32)
        for h, lo in ((0, 0), (1, 64)):
            bqt[lo : lo + 64, h] = bq[hg * CC + h * 64 : hg * CC + (h + 1) * 64]
            bkt[lo : lo + 64, h] = bk[hg * CC + h * 64 : hg * CC + (h + 1) * 64]
        for lo in (0, 64):
            bqt[lo : lo + 64, 2] = bq[hg * CC + 128 : hg * CC + 192]
            bkt[lo : lo + 64, 2] = bk[hg * CC + 128 : hg * CC + 192]
        maps.append(
            {
                "hsT": hsT,
                "hsqT": hsqT[sh],
                "wqb": qk_chunks(Wq, hg),
                "wkb": qk_chunks(Wk, hg),
                "wvb": v_chunks(wv_aug),
                "bqt": bqt,
                "bkt": bkt,
                "bvb": np.ascontiguousarray(
                    np.broadcast_to(bv_aug.astype(bf16np), (128, VC))
                ),
                "maskt": maskt,
                "ident": ident,
            }
        )
    return maps


def kernel(hidden_states, attention_mask, Wq, bq, Wk, bk, Wv, bv, **run_kwargs):
    hs = np.ascontiguousarray(np.asarray(hidden_states, np.float32).reshape(S, HID))
    mask = np.ascontiguousarray(np.asarray(attention_mask, np.float32).reshape(S))
    Wq = np.asarray(Wq, np.float32)
    Wk = np.asarray(Wk, np.float32)
    Wv = np.asarray(Wv, np.float32)
    bq = np.asarray(bq, np.float32)
    bk = np.asarray(bk, np.float32)
    bv = np.asarray(bv, np.float32)

    nc = _get_nc()
    maps = _in_maps(hs, mask, Wq, bq, Wk, bk, Wv, bv)
    res = bass_utils.run_bass_kernel_spmd(
        nc, maps, core_ids=list(range(N_CORES)), **run_kwargs
    )
    out = np.zeros((S, NH * HD), np.float32)
    for core in range(N_CORES):
        hg, sh = core // QS, core % QS
        out[sh * SQ : (sh + 1) * SQ, hg * CC : (hg + 1) * CC] = res.results[core][
            "out"
        ]
    if "trace" in run_kwargs:
        _CACHE["last_result"] = res
    return out.reshape(B, S, NH * HD)
